# revision 4
# baseline (speedup 1.0000x reference)
"""Trainium2 Bass kernel for nn_Encoder: B=1M samples through
concat(x,c) -> per-joint Linear(5,3)+ReLU -> Linear(51,32)+ReLU ->
Linear(32,16)+ReLU -> {Linear(16,3) mu, Linear(16,3) log_var},
data-parallel across 8 NeuronCores.

The kernel is HBM-bound, so the design minimizes DMA bytes and keeps
every transfer 128 partitions tall (measured DMA rate collapses below
128 rows: 128 -> ~359 GB/s, 96 -> 220, 85 -> 113 GB/s):

- Input ships bf16 FOLDED 3 samples / 2 columns of 128 rows (255/256
  slots carry real features): 170.7 B/sample instead of the 256
  B/sample a zero-padded [128, N] layout costs.
  Group g = 1536 samples = 512 A-cols + 512 B-cols:
    A-col j: rows 0:85 = feats(wholeA), rows 85:128 = feats(q)[0:43]
    B-col j: rows 0:42 = feats(q)[43:85], rows 42:127 = feats(wholeB)
- L1 per group is 3 matmuls: mmA (A-cols, M=128 stationary computing
  wholeA h1 into psum rows 0:51 AND the straddler q's partial into
  rows 64:115 in one pass), mmBacc (B-cols, start=False accumulate of
  q's other half), mmBw (B-cols -> wholeB h1, out row-half by group
  parity). Every h1 unit is then [128, 512] with one sample in rows
  0:51 and another in rows 64:115.
- L2/L3/L4 are a uniform block-diagonal funnel (shared [128, 64]
  stationaries, out row-half by unit parity, psum tiles [128, 1024]
  packing 4 units) ending in 16 out units [128, 512] (mu/lv rows
  6p..6p+5 at bases 0/64).
- psum->SBUF bias+ReLU copies are the second-scarcest resource (only
  DVE and ACT can read PSUM); they are widened to [128, 1024] where
  psum banks allow and alternated across both engines.
- NG=82 groups = 125952 padded samples/core (0.76% pad); unit streams
  are flushed with zero dummies so the 124 L2 units cascade evenly
  into 16 out units. The host unpermutes outputs via precomputed
  flat-index maps.

Measured: rel err 5.66e-3 vs the fp32 reference; per-core HBM traffic
21.5 MB in + 2.1 MB out (baseline shipped 33.6 MB in).
"""
import numpy as np
import ml_dtypes

import concourse.bass as bass
import concourse.mybir as mybir
import concourse.tile as tile
from concourse.bass_utils import run_bass_kernel_spmd

AF = mybir.ActivationFunctionType
ALU = mybir.AluOpType
F32 = mybir.dt.float32
BF16 = mybir.dt.bfloat16
BF16_NP = ml_dtypes.bfloat16

N_CORES = 8
B_FULL = 1_000_000
PER_CORE = B_FULL // N_CORES      # 125000
NG = 82                           # groups of 1536 samples per core
GSAMP = 1536
NTOT = NG * GSAMP                 # 125952 padded samples per core
NCOLS = NG * 1024
GPB = 4                           # groups per DMA block

# weight pack: 6 stationary tiles in 128-col slots
_WNAMES = ['wA', 'wBacc', 'wBw', 'w2', 'w3', 'wh']
_WCOLS = 128 * len(_WNAMES)

_ws_ctr = [0]


def _split_excess_waits(nc, max_waits=1):
    for fn in nc.m.functions:
        for bb in fn.blocks:
            insts = bb.instructions
            i = 0
            while i < len(insts):
                inst = insts[i]
                si = inst.sync_info
                if si is None or si.on_wait is None or \
                        len(si.on_wait) <= max_waits:
                    i += 1
                    continue
                waits = list(si.on_wait)
                keep = waits[-max_waits:]
                excess = waits[:-max_waits]
                new_nops = []
                for w in excess:
                    _ws_ctr[0] += 1
                    nop = mybir.InstNoOp(
                        name=f"I-waitsplit-{_ws_ctr[0]}",
                        sync_info=mybir.SyncInfo(on_wait=[w], on_update=[]),
                        bass_nofuse=True,
                        engine=inst.engine,
                    )
                    new_nops.append(nop)
                inst.sync_info = mybir.SyncInfo(
                    on_wait=keep, on_update=list(si.on_update or []))
                for j, nop in enumerate(new_nops):
                    insts.insert(i + j, nop)
                i += len(new_nops) + 1


def _w1big(W1):
    W = np.zeros((85, 51), np.float32)
    for j in range(17):
        for o in range(3):
            for k in range(3):
                W[3 * j + k, 3 * j + o] = W1[o, k]
            for k in range(2):
                W[51 + 2 * j + k, 3 * j + o] = W1[o, 3 + k]
    return W


def _host_packs(W1, b1, W2, b2, W3, b3, Wmu, bmu, Wlv, blv):
    W1b = _w1big(W1)
    w = {n: np.zeros((128, 128), np.float32) for n in _WNAMES}
    # L1 A-pass (M=128): wholeA -> rows 0:51, q partA -> rows 64:115
    w['wA'][0:85, 0:51] = W1b
    w['wA'][85:128, 64:115] = W1b[0:43]
    # L1 B-accumulate (M=128, start=False): q partB -> rows 64:115
    w['wBacc'][0:42, 64:115] = W1b[43:85]
    # L1 B-whole (M=64, out row-half by g parity)
    w['wBw'][42:127, 0:51] = W1b
    # funnel (M=64, shared weight, out row-half by unit parity)
    w['w2'][0:51, 0:32] = W2.T
    w['w2'][64:115, 32:64] = W2.T
    for t in range(4):
        w['w3'][32 * t:32 * t + 32, 16 * t:16 * t + 16] = W3.T
    Wh = np.concatenate([Wmu, Wlv], axis=0)
    for t in range(8):
        w['wh'][16 * t:16 * t + 16, 6 * t:6 * t + 6] = Wh.T
    wpack = np.concatenate([w[n] for n in _WNAMES],
                           axis=1).astype(BF16_NP)

    b1v = np.zeros((128,), np.float32)
    b1v[0:51] = np.tile(b1, 17)
    b1v[64:115] = np.tile(b1, 17)
    b2v = np.tile(b2, 4).astype(np.float32)
    b3v = np.tile(b3, 8).astype(np.float32)
    bh = np.concatenate([bmu, blv])
    bhv = np.zeros((128,), np.float32)
    bhv[0:48] = np.tile(bh, 8)
    bhv[64:112] = np.tile(bh, 8)
    bpack = np.stack([b1v, b2v, b3v, bhv], axis=1)
    return wpack, bpack


def _prep_core(x_flat, c_flat):
    n = x_flat.shape[0]
    feats = np.zeros((NTOT, 85), np.float32)
    feats[:n, 0:51] = x_flat
    feats[:n, 51:85] = c_flat
    f = feats.reshape(NG, 3, 512, 85)
    xct = np.zeros((NG, 2, 512, 128), np.float32)
    xct[:, 0, :, 0:85] = f[:, 0]
    xct[:, 0, :, 85:128] = f[:, 2, :, 0:43]
    xct[:, 1, :, 0:42] = f[:, 2, :, 43:85]
    xct[:, 1, :, 42:127] = f[:, 1]
    xct = np.ascontiguousarray(
        np.transpose(xct, (3, 0, 1, 2))).reshape(128, NCOLS)
    return xct.astype(BF16_NP)


def _unit_order():
    """L2-unit feed order. h1A-wide units cover 2 groups (pair G):
    col-half 0 = units (wholeA(2G), q(G)); col-half 1 = (wholeA(2G+1),
    ...wait: A-pass of group g puts wholeA(g) + q-of-pair in rows; the
    q rows of BOTH passes of a pair accumulate per-group. Order below
    matches the builder: per pair G: Aq(2G), Aq(2G+1), Bw(G)."""
    order = []
    for G in range(NG // 2):
        order.append(('aq', 2 * G))
        order.append(('aq', 2 * G + 1))
        order.append(('bw', G))
    order.append(('d1', 0))
    return order


def _l2_ids():
    """Per L2-unit [2, 512] sample ids (row-half a=rows 0:51,
    b=rows 64:115)."""
    ids = []
    j = np.arange(512)
    for kind, g in _unit_order():
        if kind == 'aq':
            # A-col pass of group g: wholeA(g) + q(g) (q of group g is
            # sample block base+1024; its partA from A-cols of g and
            # partB accumulated from B-cols of g)
            ids.append(np.stack([g * GSAMP + j, g * GSAMP + 1024 + j]))
        elif kind == 'bw':
            ids.append(np.stack([(2 * g) * GSAMP + 512 + j,
                                 (2 * g + 1) * GSAMP + 512 + j]))
        else:
            ids.append(np.full((2, 512), -1))
    return ids


def _out_maps():
    """flat-index maps [NTOT, 3] into out_dev [128, NU*512].

    Funnel slot order (M=64, shared weights):
      L2-unit k -> ps2[128,1024] rows 64*(k%2), cols 512*((k%4)//2)
      h2-wide m covers units 4m..4m+3; 32-row block rb, col c holds
        unit 4m + 2*(c//512) + rb//2, sample-half rb%2.
      L3-sub z = (h2-wide z//2, col-half z%2) -> ps3 rows 64*(z%2),
        cols 512*((z%4)//2); h3-wide n covers subs 4n..4n+3.
      L4-sub w = (h3-wide w//2, col-half w%2) -> ps4 rows 64*(w%2)
        (M=48: +0:48); out-unit u = ps4 u covers subs 2u, 2u+1.
    """
    ids = _l2_ids()
    n_l2 = len(ids)                           # 124
    h2w = []
    for m in range(n_l2 // 4):
        grid = np.zeros((4, 1024), np.int64)  # [32-row block, col]
        for rb in range(4):
            for ch in range(2):
                unit = ids[4 * m + 2 * ch + rb // 2]
                grid[rb, ch * 512:(ch + 1) * 512] = unit[rb % 2]
        h2w.append(grid)
    n_sub3 = len(h2w) * 2 + 2                 # 64 (incl 2 dummies)
    h3w = []
    dummy_grid = np.full((4, 1024), -1)
    for n in range(n_sub3 // 4):
        grid = np.zeros((8, 1024), np.int64)  # [16-row block, col]
        for p in range(8):
            for ch in range(2):
                z = 4 * n + 2 * ch + p // 4
                m, half = z // 2, z % 2
                src = h2w[m] if m < len(h2w) else dummy_grid
                grid[p, ch * 512:(ch + 1) * 512] = \
                    src[p % 4, half * 512:(half + 1) * 512]
        h3w.append(grid)
    nu = len(h3w)                             # 16
    ncols = nu * 512
    mu_idx = np.zeros((NTOT, 3), np.int64)
    lv_idx = np.zeros((NTOT, 3), np.int64)
    j = np.arange(512)
    for u in range(nu):
        for wh in range(2):                   # L4-sub = h3w[u] half wh
            for p in range(8):
                sids = h3w[u][p, wh * 512:(wh + 1) * 512]
                valid = sids >= 0
                row = 64 * wh + 6 * p
                col = u * 512 + j
                for o in range(3):
                    mu_idx[sids[valid], o] = (row + o) * ncols \
                        + col[valid]
                    lv_idx[sids[valid], o] = (row + 3 + o) * ncols \
                        + col[valid]
    return mu_idx, lv_idx


_OUT_MAPS = None


def _get_out_maps():
    global _OUT_MAPS
    if _OUT_MAPS is None:
        _OUT_MAPS = _out_maps()
    return _OUT_MAPS


def n_out_units():
    return (len(_unit_order()) // 4 * 2 + 2) // 2


def build_kernel(hw_loops=1, mode='full', count=False,
                 gpb=8, chunk=4096, xc_bufs=6):
    """mode: full | dmaonly | nodma | peonly | cponly."""
    nc = bass.Bass("TRN2")
    n_l2 = len(_unit_order())                 # 124
    # ps2 31 -> L3-subs 62+2 -> ps3 16 -> L4-subs 32 -> out units 16
    nu = (n_l2 // 4 * 2 + 2) // 4
    xcd = nc.dram_tensor("xct", [128, NCOLS], BF16, kind="ExternalInput")
    wd = nc.dram_tensor("wpack", [128, _WCOLS], BF16, kind="ExternalInput")
    bd = nc.dram_tensor("bpack", [128, 4], F32, kind="ExternalInput")
    od = nc.dram_tensor("out_dev", [128, nu * 512], BF16,
                        kind="ExternalOutput")
    cd = nc.dram_tensor("cnt", [1, 16], F32,
                        kind="ExternalOutput") if count else None

    with tile.TileContext(nc) as tc:
        with tc.tile_pool(name="const", bufs=1) as constp, \
             tc.tile_pool(name="xc", bufs=xc_bufs) as xcp, \
             tc.tile_pool(name="h1a", bufs=2) as h1ap, \
             tc.tile_pool(name="h1b", bufs=2) as h1bp, \
             tc.tile_pool(name="h2", bufs=2) as h2p, \
             tc.tile_pool(name="h3", bufs=2) as h3p, \
             tc.tile_pool(name="ho", bufs=2) as hop, \
             tc.tile_pool(name="psA", bufs=1, space="PSUM") as psAp, \
             tc.tile_pool(name="psB", bufs=1, space="PSUM") as psBp, \
             tc.tile_pool(name="ps2", bufs=1, space="PSUM") as ps2p, \
             tc.tile_pool(name="ps3", bufs=1, space="PSUM") as ps3p, \
             tc.tile_pool(name="ps4", bufs=1, space="PSUM") as ps4p:

            wt = constp.tile([128, _WCOLS], BF16)
            bt = constp.tile([128, 4], F32)
            nc.sync.dma_start(out=wt, in_=wd[:, :])
            nc.sync.dma_start(out=bt, in_=bd[:, :])
            W = {n: wt[:, 128 * i:128 * (i + 1)]
                 for i, n in enumerate(_WNAMES)}
            b1v = bt[:, 0:1]
            b2v = bt[:, 1:2]
            b3v = bt[:, 2:3]
            bhv = bt[:, 3:4]
            dum1 = constp.tile([128, 512], BF16)
            dum3 = constp.tile([128, 512], BF16)
            nc.vector.memset(dum1.bitcast(mybir.dt.uint32), 0)
            nc.vector.memset(dum3.bitcast(mybir.dt.uint32), 0)

            cp_ctr = [0]

            def copy_act(dst, src, bias, relu, eng=None):
                if mode == 'peonly':
                    nc.vector.memset(
                        dst[:, 0:2].bitcast(mybir.dt.uint32), 0)
                    return
                if eng is None:
                    eng = ('dve', 'act')[cp_ctr[0] % 2]
                    cp_ctr[0] += 1
                if eng == 'act':
                    nc.scalar.activation(dst, src,
                                         AF.Relu if relu else AF.Identity,
                                         bias=bias)
                else:
                    nc.vector.tensor_scalar(
                        out=dst, in0=src, scalar1=bias, scalar2=0.0,
                        op0=ALU.add,
                        op1=ALU.max if relu else ALU.bypass)

            def mm(out, lhsT, rhs, start, stop=True):
                if mode == 'cponly':
                    return
                nc.tensor.matmul(out, lhsT, rhs, start=start, stop=stop,
                                 skip_group_check=True)

            st = {'k': 0, 'z': 0, 'w': 0, 'u': 0,
                  'ps2': None, 'ps3': None, 'ps4': None,
                  'h2': None, 'h3': None, 'hob': None}

            w2s = W['w2'][:, 0:64]
            w3s = W['w3'][:, 0:64]
            whs = W['wh'][:, 0:64]

            def l2_feed(h1ap_):
                k = st['k']
                st['k'] += 1
                if k % 4 == 0:
                    st['ps2'] = ps2p.tile([128, 1024], F32, name='ps2')
                r0 = 64 * (k % 2)
                c0 = 512 * ((k % 4) // 2)
                mm(st['ps2'][r0:r0 + 64, c0:c0 + 512], w2s, h1ap_,
                   start=True)
                if k % 4 != 3:
                    return
                st['h2'] = h2p.tile([128, 1024], BF16, name='h2')
                copy_act(st['h2'], st['ps2'], b2v, True)
                for zh in range(2):
                    l3_feed(st['h2'][:, 512 * zh:512 * zh + 512])

            def l3_feed(h2sub):
                z = st['z']
                st['z'] += 1
                if z % 4 == 0:
                    st['ps3'] = ps3p.tile([128, 1024], F32, name='ps3')
                r0 = 64 * (z % 2)
                c0 = 512 * ((z % 4) // 2)
                mm(st['ps3'][r0:r0 + 64, c0:c0 + 512], w3s, h2sub,
                   start=True)
                if z % 4 != 3:
                    return
                st['h3'] = h3p.tile([128, 1024], BF16, name='h3')
                copy_act(st['h3'], st['ps3'], b3v, True)
                for wh_ in range(2):
                    l4_feed(st['h3'][:, 512 * wh_:512 * wh_ + 512])

            def l4_feed(h3sub):
                w_ = st['w']
                st['w'] += 1
                if w_ % 2 == 0:
                    st['ps4'] = ps4p.tile([128, 512], F32, name='ps4')
                r0 = 64 * (w_ % 2)
                mm(st['ps4'][r0:r0 + 64, :], whs, h3sub, start=True)
                if w_ % 2 != 1:
                    return
                u = st['u']
                st['u'] += 1
                if u % 2 == 0:
                    st['hob'] = hop.tile([128, 1024], BF16, name='hob')
                copy_act(st['hob'][:, 512 * (u % 2):512 * (u % 2) + 512],
                         st['ps4'], bhv, False)
                if u % 2 == 1:
                    nc.sync.dma_start(
                        out=od[:, (u - 1) * 512:(u + 1) * 512],
                        in_=st['hob'])

            def body():
                st.update({'k': 0, 'z': 0, 'w': 0, 'u': 0})
                cp_ctr[0] = 0
                psA = psB = None
                nblk = (NG + gpb - 1) // gpb
                for b in range(nblk):
                    g0 = b * gpb
                    gn = min(gpb, NG - g0)
                    xcb = xcp.tile([128, gpb * 1024], BF16, tag="xcb")
                    if mode != 'nodma':
                        nch = (gn * 1024 + chunk - 1) // chunk
                        for q in range(nch):
                            c0 = q * chunk
                            c1 = min((q + 1) * chunk, gn * 1024)
                            nc.sync.dma_start(
                                out=xcb[:, c0:c1],
                                in_=xcd[:, g0 * 1024 + c0:
                                        g0 * 1024 + c1])
                    else:
                        nc.vector.memset(
                            xcb[:, 0:2].bitcast(mybir.dt.uint32), 0)
                    if mode == 'dmaonly':
                        hob = hop.tile([128, 1024], BF16, tag="hob")
                        nc.vector.tensor_copy(hob[:, 0:2], xcb[:, 0:2])
                        nc.sync.dma_start(
                            out=od[:, (b % 8) * 1024:(b % 8) * 1024
                                   + 1024],
                            in_=hob)
                        continue
                    for gi in range(gn):
                        g = g0 + gi
                        ac = xcb[:, gi * 1024:gi * 1024 + 512]
                        bc = xcb[:, gi * 1024 + 512:(gi + 1) * 1024]
                        if g % 2 == 0:
                            psA = psAp.tile([128, 1024], F32, tag="psA")
                        ch = 512 * (g % 2)
                        mm(psA[:, ch:ch + 512], W['wA'], ac, start=True,
                           stop=False)
                        mm(psA[:, ch:ch + 512], W['wBacc'], bc,
                           start=False)
                        if g % 2 == 0:
                            psB = psBp.tile([128, 512], F32, tag="psB")
                        rB = 64 * (g % 2)
                        mm(psB[rB:rB + 64, :], W['wBw'][:, 0:64], bc,
                           start=True)
                        if g % 2 == 1:
                            h1a = h1ap.tile([128, 1024], BF16,
                                            tag="h1a")
                            copy_act(h1a, psA, b1v, True)
                            h1b = h1bp.tile([128, 512], BF16, tag="h1b")
                            copy_act(h1b, psB, b1v, True)
                            l2_feed(h1a[:, 0:512])
                            l2_feed(h1a[:, 512:1024])
                            l2_feed(h1b)
                if mode == 'dmaonly':
                    return
                l2_feed(dum1)
                l3_feed(dum3)
                l3_feed(dum3)

            cnt_t = None
            if count:
                cnt_t = constp.tile([1, 16], F32)
                nc.vector.memset(cnt_t, 0)
            if hw_loops > 1:
                hint = (mybir.EngineType.PE, mybir.EngineType.DVE,
                        mybir.EngineType.Activation, mybir.EngineType.SP,
                        mybir.EngineType.Pool)
                with tc.For_i(0, hw_loops, 1, hint_engines=hint):
                    body()
                    if count:
                        nc.vector.tensor_scalar(
                            out=cnt_t, in0=cnt_t, scalar1=1.0,
                            scalar2=0.0, op0=ALU.add, op1=ALU.add)
            else:
                body()
            if count:
                nc.sync.dma_start(out=cd[:, :], in_=cnt_t)

    _split_excess_waits(nc)
    return nc


def make_input_map(rng):
    return {
        "xct": rng.standard_normal((128, NCOLS), np.float32)
               .astype(BF16_NP),
        "wpack": rng.standard_normal((128, _WCOLS), np.float32)
                 .astype(BF16_NP),
        "bpack": rng.standard_normal((128, 4)).astype(np.float32),
    }


_NC_CACHE = {}


def _get_nc(key=(1, 'full')):
    if key not in _NC_CACHE:
        _NC_CACHE[key] = build_kernel(*key)
    return _NC_CACHE[key]


def kernel(x, c, W1, b1, W2, b2, W3, b3, Wmu, bmu, Wlv, blv, _trace=False):
    x = np.asarray(x, np.float32).reshape(B_FULL, 51)
    c = np.asarray(c, np.float32).reshape(B_FULL, 34)
    wpack, bpack = _host_packs(
        np.asarray(W1, np.float32), np.asarray(b1, np.float32),
        np.asarray(W2, np.float32), np.asarray(b2, np.float32),
        np.asarray(W3, np.float32), np.asarray(b3, np.float32),
        np.asarray(Wmu, np.float32), np.asarray(bmu, np.float32),
        np.asarray(Wlv, np.float32), np.asarray(blv, np.float32))

    in_maps = []
    for core in range(N_CORES):
        sl = slice(core * PER_CORE, (core + 1) * PER_CORE)
        in_maps.append({"xct": _prep_core(x[sl], c[sl]),
                        "wpack": wpack, "bpack": bpack})

    nc = _get_nc()
    res = run_bass_kernel_spmd(nc, in_maps, core_ids=list(range(N_CORES)),
                               trace=_trace)
    mu_idx, lv_idx = _get_out_maps()
    mus, lvs = [], []
    for i in range(N_CORES):
        flat = np.asarray(res.results[i]["out_dev"],
                          dtype=np.float32).ravel()
        mus.append(flat[mu_idx][:PER_CORE])
        lvs.append(flat[lv_idx][:PER_CORE])
    out = (np.concatenate(mus), np.concatenate(lvs))
    if _trace:
        return out, res
    return out
